# revision 32
# baseline (speedup 1.0000x reference)
"""Trainium2 Bass kernel for nn_Capsule: capsule layer with 3 dynamic-routing
iterations.

    u_hat = einsum('bip,iodp->biod', x, W)   # [64, 2048, 32, 32]
    3x routing: c = softmax(b, axis=2); s = sum_i c*u_hat; v = squash(s);
                b += sum_d v*u_hat

Strategy: shard in_caps (i) across 8 cores (256 each). W-shard and a
block-diagonalized x are SBUF-resident; u_hat is (re)computed on the tensor
engine each routing iteration, two capsules at a time, as
[K=32 (2i x 16p), M=128 (2i x 64b)] x [K=32, N=512 od] matmuls with
block-diagonal x as the stationary operand, spread over 4 PE row-groups.
Iteration 0 (uniform c) accumulates u directly in PSUM. Iterations 1-2
consume u tiles from PSUM on the vector engine: agreement = reduce_d(u*v),
logits update + softmax (ACT exp), s += c*u. Per-core s partials are
AllReduced after iters 0 and 1; the final iteration's partials are summed
and squashed on the host.
"""

import numpy as np

B, IN_CAPS, IN_DIM = 64, 2048, 16
NUM_CAPS, DIM_CAPS = 32, 32
OD = NUM_CAPS * DIM_CAPS  # 1024
ROUTING_ITERS = 3
EPS = 1e-7

N_CORES = 8
I_LOC = IN_CAPS // N_CORES       # 256
N_PAIRS = I_LOC // 2             # 128
N_RG = 4                         # PE row groups
N_PJ = N_PAIRS // N_RG           # 32 pairs per row group

_CACHE = {}


def _build_nc(stage=3):
    # stage 0: iter0 partial s0 only; 1: +AllReduce+squash (out=vrep);
    # 2: +iter1 (out=acc, no 2nd AllReduce); 3: full kernel.
    import concourse.bacc as bacc
    import concourse.bass as bass
    import concourse.tile as tile
    from concourse import mybir

    f32 = mybir.dt.float32
    Alu = mybir.AluOpType
    Act = mybir.ActivationFunctionType
    AxX = mybir.AxisListType.X

    nc = bacc.Bacc("TRN2", target_bir_lowering=False, debug=False,
                   num_devices=N_CORES)

    bf16 = mybir.dt.bfloat16
    xbd_d = nc.dram_tensor("xbd", [128, N_PJ * 128], bf16,
                           kind="ExternalInput")
    wp_d = nc.dram_tensor("wp", [128, N_PJ * OD], bf16, kind="ExternalInput")
    out_d = nc.dram_tensor("out_sp2", [128, OD], f32, kind="ExternalOutput")

    def bcast_d(ap_2d, o_cnt):
        """View a [P, o_cnt] AP as [P, o_cnt, DIM_CAPS] with the last dim
        broadcast (step 0)."""
        return bass.AP(tensor=ap_2d.tensor, offset=ap_2d.offset,
                       ap=[list(ap_2d.ap[0]), list(ap_2d.ap[1]),
                           [0, DIM_CAPS]])

    def lhsT_of(xbd, rg, pj):
        return xbd[32 * rg:32 * rg + 32, 128 * pj:128 * pj + 128]

    def rhs_of(wp, rg, pj, h):
        return wp[32 * rg:32 * rg + 32,
                  OD * pj + 512 * h:OD * pj + 512 * h + 512]

    with tile.TileContext(nc) as tc:
        with (
            nc.allow_low_precision(reason="bf16 routing intermediates"),
            tc.tile_pool(name="big", bufs=1) as big,
            tc.tile_pool(name="work", bufs=3) as work,
            tc.tile_pool(name="small", bufs=1) as small,
            tc.tile_pool(name="dram", bufs=1, space="DRAM") as dram,
        ):
            xbd = big.tile([128, N_PJ * 128], bf16)
            wp = big.tile([128, N_PJ * OD], bf16)
            for rg in range(N_RG):
                sl = slice(32 * rg, 32 * rg + 32)
                nc.sync.dma_start(xbd[sl, :], xbd_d[sl, :])
                nc.sync.dma_start(wp[sl, :], wp_d[sl, :])

            bl = big.tile([128, N_PAIRS * NUM_CAPS], bf16)  # routing logits
            acc = big.tile([128, OD], f32)                  # folded s partial
            acc2 = big.tile([128, 2 * OD], f32)             # quad acc (DVE)
            acc2_g = big.tile([128, 2 * OD], f32)           # quad acc (GPSIMD)
            vrep = big.tile([128, OD], bf16)                # v replicated 2x
            eps_t = big.tile([64, 1], f32)
            nc.vector.memset(eps_t[:], EPS)

            ar_count = [0]

            # ---------------- iteration 0: s0 = (1/32) * sum_i u ----------
            with tc.tile_pool(name="ps0", bufs=1, space="PSUM") as ps0:
                acc0 = [[ps0.tile([128, 512], f32, name=f"acc0_{rg}_{h}",
                                  tag=f"acc0_{rg}_{h}")
                         for h in range(2)] for rg in range(N_RG)]
                for pj in range(N_PJ):
                    for h in range(2):
                        for rg in range(N_RG):
                            nc.tensor.matmul(
                                acc0[rg][h][:],
                                lhsT_of(xbd, rg, pj),
                                rhs_of(wp, rg, pj, h),
                                start=(pj == 0), stop=(pj == N_PJ - 1),
                                tile_position=(32 * rg, 0),
                            )
                # fold row groups: t[h] = sum_rg acc0[rg][h]  (on SBUF)
                s0 = small.tile([64, OD], f32, tag="sfold")
                tmpu = small.tile([64, OD], f32, tag="tmpu")
                for h in range(2):
                    th = work.tile([128, 512], f32, tag="thfold")
                    nc.scalar.copy(out=th[:], in_=acc0[0][h][:])
                    for rg in range(1, N_RG):
                        nc.vector.tensor_add(out=th[:], in0=th[:],
                                             in1=acc0[rg][h][:])
                    # fold the two capsule slots (partitions 0-63 + 64-127);
                    # DVE can't read two different base partitions, so shift
                    # the upper half down via SBUF->SBUF DMA first.
                    nc.sync.dma_start(tmpu[:, 512 * h:512 * h + 512],
                                      th[64:128, :])
                    nc.vector.tensor_add(out=s0[:, 512 * h:512 * h + 512],
                                         in0=th[0:64, :],
                                         in1=tmpu[:, 512 * h:512 * h + 512])
            # scale by 1/NUM_CAPS (uniform softmax weight)
            nc.scalar.mul(out=s0[:], in_=s0[:], mul=1.0 / NUM_CAPS)
            if stage == 0:
                nc.sync.dma_start(out_d[0:64, :], s0[:])

            def all_reduce(sp):
                if stage == 4:  # timing variant: skip collectives
                    return sp
                k = ar_count[0]
                ar_count[0] += 1
                ar_in = dram.tile([64, OD], f32, name=f"ar_in{k}",
                                  tag=f"ar_in{k}")
                ar_out = dram.tile([64, OD], f32, name=f"ar_out{k}",
                                   tag=f"ar_out{k}")
                nc.sync.dma_start(ar_in[:], sp[:])
                nc.gpsimd.collective_compute(
                    "AllReduce", Alu.add,
                    replica_groups=[list(range(N_CORES))],
                    ins=[ar_in.opt()], outs=[ar_out.opt()])
                sq = small.tile([64, OD], f32, tag="sfold")
                nc.sync.dma_start(sq[:], ar_out[:])
                return sq

            def squash_to_vrep(sq):
                """v = (n/(1+n)) * s / sqrt(n+eps), n = sum_d s^2; then
                replicate v into both partition halves of vrep."""
                ssq = small.tile([64, OD], f32, tag="tmpu")
                nc.vector.tensor_mul(out=ssq[:], in0=sq[:], in1=sq[:])
                n_t = small.tile([64, NUM_CAPS], f32, tag="n_t")
                # d-major layout: reduce over d (strided, AP dims [o, d])
                nc.vector.tensor_reduce(
                    out=n_t[:],
                    in_=bass.AP(tensor=ssq.tensor, offset=ssq[:].offset,
                                ap=[list(ssq[:].ap[0]), [1, NUM_CAPS],
                                    [NUM_CAPS, DIM_CAPS]]),
                    axis=AxX, op=Alu.add)
                sr = small.tile([64, NUM_CAPS], f32, tag="sr")
                # sqrt via exp(0.5*ln): Ln/Exp share an ACT table set, so no
                # mid-kernel table reloads (Sqrt lives in a different set)
                nc.scalar.activation(out=sr[:], in_=n_t[:], func=Act.Ln,
                                     bias=eps_t[:], scale=1.0)
                nc.scalar.activation(out=sr[:], in_=sr[:], func=Act.Exp,
                                     bias=0.0, scale=0.5)
                nc.vector.reciprocal(out=sr[:], in_=sr[:])   # 1/sqrt(n+eps)
                np1 = small.tile([64, NUM_CAPS], f32, tag="np1")
                nc.vector.tensor_scalar_add(out=np1[:], in0=n_t[:],
                                            scalar1=1.0)
                nc.vector.reciprocal(out=np1[:], in_=np1[:])  # 1/(1+n)
                fac = small.tile([64, NUM_CAPS], f32, tag="fac")
                nc.vector.tensor_mul(out=fac[:], in0=n_t[:], in1=np1[:])
                nc.vector.tensor_mul(out=fac[:], in0=fac[:], in1=sr[:])
                # v = s * fac (broadcast fac over the outer d dim)
                nc.vector.tensor_tensor(
                    out=vrep[0:64, :].rearrange("p (d o) -> p d o",
                                                d=DIM_CAPS),
                    in0=sq[:].rearrange("p (d o) -> p d o", d=DIM_CAPS),
                    in1=bass.AP(tensor=fac.tensor, offset=fac[:].offset,
                                ap=[list(fac[:].ap[0]), [0, DIM_CAPS],
                                    [1, NUM_CAPS]]),
                    op=Alu.mult)
                nc.sync.dma_start(vrep[64:128, :], vrep[0:64, :])

            if stage >= 1:
                sq = all_reduce(s0)
                squash_to_vrep(sq)
            if stage == 1:
                vr32 = work.tile([128, OD], f32, tag="vr32")
                nc.scalar.copy(out=vr32[:], in_=vrep[:])
                nc.sync.dma_start(out_d[:], vr32[:])

            # ---------------- iterations 1..2 -----------------------------
            last_it = ROUTING_ITERS if stage >= 3 else stage
            if stage == 4:
                last_it = ROUTING_ITERS
            with tc.tile_pool(name="ps", bufs=4, space="PSUM") as ps:
                N_QUADS = N_PAIRS // 2
                GRP = 4  # quads per accumulation group (bf16 add tree)
                for it in range(1, last_it):
                    nc.gpsimd.memset(acc2[:], 0.0)
                    nc.gpsimd.memset(acc2_g[:], 0.0)
                    cm_tiles = {0: [], 1: []}
                    for q in range(N_QUADS):
                        # a quad = 2 consecutive pairs (4 capsules);
                        # alternate quads between DVE and GPSIMD streams
                        side = q % 2
                        eng = nc.vector if side == 0 else nc.gpsimd
                        my_acc = acc2 if side == 0 else acc2_g
                        ub = work.tile([128, 2 * OD], bf16, tag="ub", bufs=8)
                        for sub in range(2):
                            pair = 2 * q + sub
                            rg, pj = pair % N_RG, pair // N_RG
                            ups = ps.tile([128, OD], f32, name="ups",
                                          tag="ups")
                            for h in range(2):
                                nc.tensor.matmul(
                                    ups[:, 512 * h:512 * h + 512],
                                    lhsT_of(xbd, rg, pj),
                                    rhs_of(wp, rg, pj, h),
                                    start=True, stop=True,
                                    tile_position=(32 * rg, 0),
                                )
                            # evacuate u to SBUF as bf16 on the scalar
                            # engine so DVE tensor ops run in 2x mode
                            nc.scalar.copy(out=ub[:, OD * sub:OD * (sub + 1)],
                                           in_=ups[:])
                        # agreement = sum_d u * v  (both pairs at once);
                        # free layout of u is (sub, d, o) -- d-major
                        m = work.tile([128, 2 * OD], bf16, tag="m", bufs=2)
                        nc.vector.tensor_tensor(
                            out=m[:].rearrange("p (s od) -> p s od", s=2),
                            in0=ub[:].rearrange("p (s od) -> p s od", s=2),
                            in1=bass.AP(tensor=vrep.tensor,
                                        offset=vrep[:].offset,
                                        ap=[list(vrep[:].ap[0]), [0, 2],
                                            [1, OD]]),
                            op=Alu.mult)
                        # two halvings over d (2x bf16 adds on contiguous
                        # d-blocks), then a strided reduce over the rest
                        mh = work.tile([128, OD], bf16, tag="mh", bufs=3)
                        mv = m[:].rearrange("p (s hd x) -> p s hd x",
                                            s=2, hd=2)
                        nc.vector.tensor_tensor(
                            out=mh[:].rearrange("p (s x) -> p s x", s=2),
                            in0=mv[:, :, 0, :], in1=mv[:, :, 1, :],
                            op=Alu.add)
                        mhv = mh[:].rearrange("p (s hd x) -> p s hd x",
                                              s=2, hd=2)
                        nc.vector.tensor_tensor(
                            out=mhv[:, :, 0, :], in0=mhv[:, :, 0, :],
                            in1=mhv[:, :, 1, :], op=Alu.add)
                        # reduce remaining 8 d-blocks: AP dims [s, o, d]
                        red_in = bass.AP(
                            tensor=mh.tensor, offset=mh[:].offset,
                            ap=[list(mh[:].ap[0]), [512, 2], [1, NUM_CAPS],
                                [NUM_CAPS, 8]])
                        bsl = bl[:, NUM_CAPS * 2 * q:NUM_CAPS * 2 * (q + 1)]
                        if it == 1:
                            # b was zero: logits = agreement, written directly
                            nc.vector.tensor_reduce(
                                out=bsl, in_=red_in, axis=AxX, op=Alu.add)
                        else:
                            agr = work.tile([128, 2 * NUM_CAPS], bf16,
                                            tag="agr", bufs=6)
                            nc.vector.tensor_reduce(
                                out=agr[:], in_=red_in, axis=AxX, op=Alu.add)
                            nc.vector.tensor_add(out=bsl, in0=bsl,
                                                 in1=agr[:])
                        # softmax over o (free dim); logits are small, so
                        # exp without max-subtraction is safe
                        ce = work.tile([128, 2 * NUM_CAPS], bf16, tag="ce", bufs=10)
                        zs = work.tile([128, 2], f32, tag="zs", bufs=10)
                        nc.scalar.activation(
                            out=ce[:].rearrange("p (s o) -> p s o", s=2),
                            in_=bsl.rearrange("p (s o) -> p s o", s=2),
                            func=Act.Exp)
                        nc.vector.tensor_reduce(
                            out=zs[:],
                            in_=ce[:].rearrange("p (s o) -> p s o", s=2),
                            axis=AxX, op=Alu.add)
                        nc.vector.reciprocal(out=zs[:], in_=zs[:])
                        # c = e / Z  (broadcast 1/Z over o)
                        nc.vector.tensor_tensor(
                            out=ce[:].rearrange("p (s o) -> p s o", s=2),
                            in0=ce[:].rearrange("p (s o) -> p s o", s=2),
                            in1=bass.AP(tensor=zs.tensor, offset=zs[:].offset,
                                        ap=[list(zs[:].ap[0]), [1, 2],
                                            [0, NUM_CAPS]]),
                            op=Alu.mult)
                        # cm = c * u  (c broadcast over the outer d dim ->
                        # innermost step stays 1, keeps 2x mode)
                        csl = bass.AP(
                            tensor=ce.tensor, offset=ce[:].offset,
                            ap=[list(ce[:].ap[0]), [NUM_CAPS, 2],
                                [0, DIM_CAPS], [1, NUM_CAPS]])
                        cm = work.tile([128, 2 * OD], bf16, name="cm",
                                       tag="cm", bufs=8)
                        eng.tensor_tensor(
                            out=cm[:].rearrange("p (s d o) -> p s d o",
                                                s=2, d=DIM_CAPS),
                            in0=ub[:].rearrange("p (s d o) -> p s d o",
                                                s=2, d=DIM_CAPS),
                            in1=csl, op=Alu.mult)
                        cm_tiles[side].append(cm)
                        if len(cm_tiles[side]) == GRP:
                            c0, c1, c2, c3 = cm_tiles[side]
                            cm_tiles[side] = []
                            eng.tensor_add(out=c0[:], in0=c0[:], in1=c1[:])
                            eng.tensor_add(out=c2[:], in0=c2[:], in1=c3[:])
                            eng.tensor_add(out=c0[:], in0=c0[:], in1=c2[:])
                            eng.tensor_add(out=my_acc[:], in0=my_acc[:],
                                           in1=c0[:])
                    # fold acc2/acc2_g [128, 2*OD] f32 into acc [128, OD]
                    nc.vector.tensor_add(out=acc[:], in0=acc2[:, 0:OD],
                                         in1=acc2[:, OD:2 * OD])
                    nc.vector.tensor_add(out=acc[:], in0=acc[:],
                                         in1=acc2_g[:, 0:OD])
                    nc.vector.tensor_add(out=acc[:], in0=acc[:],
                                         in1=acc2_g[:, OD:2 * OD])
                    if it < last_it - 1:
                        sp = small.tile([64, OD], f32, tag="sfold")
                        tmpu2 = small.tile([64, OD], f32, tag="tmpu")
                        nc.sync.dma_start(tmpu2[:], acc[64:128, :])
                        nc.vector.tensor_add(out=sp[:], in0=acc[0:64, :],
                                             in1=tmpu2[:])
                        sq = all_reduce(sp)
                        squash_to_vrep(sq)
                    else:
                        nc.sync.dma_start(out_d[:], acc[:])
    nc.compile()
    return nc


def _prep_inputs(x, W):
    """Build per-core xbd [128, N_PJ*128] and wp [128, N_PJ*OD] arrays."""
    import ml_dtypes
    bf16 = ml_dtypes.bfloat16
    ins = []
    for c in range(N_CORES):
        xc = x[:, c * I_LOC:(c + 1) * I_LOC, :]          # [64, 256, 16]
        Wc = W[c * I_LOC:(c + 1) * I_LOC]                # [256, 32, 32, 16]
        # i_loc = 8*pj + 2*rg + ipar
        xr = np.ascontiguousarray(
            xc.reshape(B, N_PJ, N_RG, 2, IN_DIM)
              .transpose(3, 2, 4, 1, 0))                 # [ipar,rg,p,pj,b]
        xbd = np.zeros((N_RG, 2, IN_DIM, N_PJ, 2, B), dtype=np.float32)
        xbd[:, 0, :, :, 0, :] = xr[0]
        xbd[:, 1, :, :, 1, :] = xr[1]
        xbd = xbd.reshape(128, N_PJ * 128).astype(bf16)
        wr = np.ascontiguousarray(
            Wc.reshape(N_PJ, N_RG, 2, NUM_CAPS, DIM_CAPS, IN_DIM)
              .transpose(1, 2, 5, 0, 4, 3)               # [rg,ipar,p,pj,d,o]
              .reshape(128, N_PJ * OD)).astype(bf16)
        ins.append({"xbd": xbd, "wp": wr})
    return ins


def _squash_np(s):
    n = np.sum(np.square(s), axis=-1, keepdims=True)
    return (n / (1.0 + n)) * (s / np.sqrt(n + EPS))


def kernel(x, W, _trace=False):
    from concourse.bass_utils import run_bass_kernel_spmd

    x = np.asarray(x, dtype=np.float32)
    W = np.asarray(W, dtype=np.float32)
    if "nc" not in _CACHE:
        _CACHE["nc"] = _build_nc()
    nc = _CACHE["nc"]
    in_maps = _prep_inputs(x, W)
    res = run_bass_kernel_spmd(nc, in_maps, core_ids=list(range(N_CORES)),
                               trace=_trace)
    _CACHE["last_result"] = res
    sp = np.stack([r["out_sp2"] for r in res.results])   # [8, 128, OD]
    s2 = sp[:, 0:64, :].sum(axis=0) + sp[:, 64:128, :].sum(axis=0)
    s2_od = s2.reshape(B, DIM_CAPS, NUM_CAPS).transpose(0, 2, 1)
    v = _squash_np(np.ascontiguousarray(s2_od))
    return v.astype(np.float32)


# revision 33
# speedup vs baseline: 1.0369x; 1.0369x over previous
"""Trainium2 Bass kernel for nn_Capsule: capsule layer with 3 dynamic-routing
iterations.

    u_hat = einsum('bip,iodp->biod', x, W)   # [64, 2048, 32, 32]
    3x routing: c = softmax(b, axis=2); s = sum_i c*u_hat; v = squash(s);
                b += sum_d v*u_hat

Strategy: shard in_caps (i) across 8 cores (256 each). W-shard and a
block-diagonalized x are SBUF-resident; u_hat is (re)computed on the tensor
engine each routing iteration, two capsules at a time, as
[K=32 (2i x 16p), M=128 (2i x 64b)] x [K=32, N=512 od] matmuls with
block-diagonal x as the stationary operand, spread over 4 PE row-groups.
Iteration 0 (uniform c) accumulates u directly in PSUM. Iterations 1-2
consume u tiles from PSUM on the vector engine: agreement = reduce_d(u*v),
logits update + softmax (ACT exp), s += c*u. Per-core s partials are
AllReduced after iters 0 and 1; the final iteration's partials are summed
and squashed on the host.
"""

import numpy as np

B, IN_CAPS, IN_DIM = 64, 2048, 16
NUM_CAPS, DIM_CAPS = 32, 32
OD = NUM_CAPS * DIM_CAPS  # 1024
ROUTING_ITERS = 3
EPS = 1e-7

N_CORES = 8
I_LOC = IN_CAPS // N_CORES       # 256
N_PAIRS = I_LOC // 2             # 128
N_RG = 4                         # PE row groups
N_PJ = N_PAIRS // N_RG           # 32 pairs per row group

_CACHE = {}


def _build_nc(stage=3):
    # stage 0: iter0 partial s0 only; 1: +AllReduce+squash (out=vrep);
    # 2: +iter1 (out=acc, no 2nd AllReduce); 3: full kernel.
    import concourse.bacc as bacc
    import concourse.bass as bass
    import concourse.tile as tile
    from concourse import mybir

    f32 = mybir.dt.float32
    Alu = mybir.AluOpType
    Act = mybir.ActivationFunctionType
    AxX = mybir.AxisListType.X

    nc = bacc.Bacc("TRN2", target_bir_lowering=False, debug=False,
                   num_devices=N_CORES)

    bf16 = mybir.dt.bfloat16
    xbd_d = nc.dram_tensor("xbd", [128, N_PJ * 128], bf16,
                           kind="ExternalInput")
    wp_d = nc.dram_tensor("wp", [128, N_PJ * OD], bf16, kind="ExternalInput")
    out_d = nc.dram_tensor("out_sp2", [128, OD], f32, kind="ExternalOutput")

    def bcast_d(ap_2d, o_cnt):
        """View a [P, o_cnt] AP as [P, o_cnt, DIM_CAPS] with the last dim
        broadcast (step 0)."""
        return bass.AP(tensor=ap_2d.tensor, offset=ap_2d.offset,
                       ap=[list(ap_2d.ap[0]), list(ap_2d.ap[1]),
                           [0, DIM_CAPS]])

    def lhsT_of(xbd, rg, pj):
        return xbd[32 * rg:32 * rg + 32, 128 * pj:128 * pj + 128]

    def rhs_of(wp, rg, pj, h):
        return wp[32 * rg:32 * rg + 32,
                  OD * pj + 512 * h:OD * pj + 512 * h + 512]

    with tile.TileContext(nc) as tc:
        with (
            nc.allow_low_precision(reason="bf16 routing intermediates"),
            tc.tile_pool(name="big", bufs=1) as big,
            tc.tile_pool(name="work", bufs=3) as work,
            tc.tile_pool(name="small", bufs=1) as small,
            tc.tile_pool(name="dram", bufs=1, space="DRAM") as dram,
        ):
            xbd = big.tile([128, N_PJ * 128], bf16)
            wp = big.tile([128, N_PJ * OD], bf16)
            for rg in range(N_RG):
                sl = slice(32 * rg, 32 * rg + 32)
                nc.sync.dma_start(xbd[sl, :], xbd_d[sl, :])
                nc.sync.dma_start(wp[sl, :], wp_d[sl, :])

            bl = big.tile([128, N_PAIRS * NUM_CAPS], bf16)  # routing logits
            acc = big.tile([128, OD], f32)                  # folded s partial
            acc2 = big.tile([128, 2 * OD], f32)             # quad acc (DVE)
            acc2_g = big.tile([128, 2 * OD], f32)           # quad acc (GPSIMD)
            vrep = big.tile([128, OD], bf16)                # v replicated 2x
            eps_t = big.tile([64, 1], f32)
            nc.vector.memset(eps_t[:], EPS)

            ar_count = [0]

            # ---------------- iteration 0: s0 = (1/32) * sum_i u ----------
            with tc.tile_pool(name="ps0", bufs=1, space="PSUM") as ps0:
                acc0 = [[ps0.tile([128, 512], f32, name=f"acc0_{rg}_{h}",
                                  tag=f"acc0_{rg}_{h}")
                         for h in range(2)] for rg in range(N_RG)]
                for pj in range(N_PJ):
                    for h in range(2):
                        for rg in range(N_RG):
                            nc.tensor.matmul(
                                acc0[rg][h][:],
                                lhsT_of(xbd, rg, pj),
                                rhs_of(wp, rg, pj, h),
                                start=(pj == 0), stop=(pj == N_PJ - 1),
                                tile_position=(32 * rg, 0),
                            )
                # fold row groups: t[h] = sum_rg acc0[rg][h]  (on SBUF)
                s0 = small.tile([64, OD], f32, tag="sfold")
                tmpu = small.tile([64, OD], f32, tag="tmpu")
                for h in range(2):
                    th = work.tile([128, 512], f32, tag="thfold")
                    nc.scalar.copy(out=th[:], in_=acc0[0][h][:])
                    for rg in range(1, N_RG):
                        nc.vector.tensor_add(out=th[:], in0=th[:],
                                             in1=acc0[rg][h][:])
                    # fold the two capsule slots (partitions 0-63 + 64-127);
                    # DVE can't read two different base partitions, so shift
                    # the upper half down via SBUF->SBUF DMA first.
                    nc.sync.dma_start(tmpu[:, 512 * h:512 * h + 512],
                                      th[64:128, :])
                    nc.vector.tensor_add(out=s0[:, 512 * h:512 * h + 512],
                                         in0=th[0:64, :],
                                         in1=tmpu[:, 512 * h:512 * h + 512])
            # scale by 1/NUM_CAPS (uniform softmax weight)
            nc.scalar.mul(out=s0[:], in_=s0[:], mul=1.0 / NUM_CAPS)
            if stage == 0:
                nc.sync.dma_start(out_d[0:64, :], s0[:])

            def all_reduce(sp):
                if stage == 4:  # timing variant: skip collectives
                    return sp
                k = ar_count[0]
                ar_count[0] += 1
                ar_in = dram.tile([64, OD], f32, name=f"ar_in{k}",
                                  tag=f"ar_in{k}")
                ar_out = dram.tile([64, OD], f32, name=f"ar_out{k}",
                                   tag=f"ar_out{k}")
                nc.sync.dma_start(ar_in[:], sp[:])
                nc.gpsimd.collective_compute(
                    "AllReduce", Alu.add,
                    replica_groups=[list(range(N_CORES))],
                    ins=[ar_in.opt()], outs=[ar_out.opt()])
                sq = small.tile([64, OD], f32, tag="sfold")
                nc.sync.dma_start(sq[:], ar_out[:])
                return sq

            def squash_to_vrep(sq):
                """v = (n/(1+n)) * s / sqrt(n+eps), n = sum_d s^2; then
                replicate v into both partition halves of vrep."""
                ssq = small.tile([64, OD], f32, tag="tmpu")
                nc.vector.tensor_mul(out=ssq[:], in0=sq[:], in1=sq[:])
                n_t = small.tile([64, NUM_CAPS], f32, tag="n_t")
                # d-major layout: reduce over d (strided, AP dims [o, d])
                nc.vector.tensor_reduce(
                    out=n_t[:],
                    in_=bass.AP(tensor=ssq.tensor, offset=ssq[:].offset,
                                ap=[list(ssq[:].ap[0]), [1, NUM_CAPS],
                                    [NUM_CAPS, DIM_CAPS]]),
                    axis=AxX, op=Alu.add)
                sr = small.tile([64, NUM_CAPS], f32, tag="sr")
                # sqrt via exp(0.5*ln): Ln/Exp share an ACT table set, so no
                # mid-kernel table reloads (Sqrt lives in a different set)
                nc.scalar.activation(out=sr[:], in_=n_t[:], func=Act.Ln,
                                     bias=eps_t[:], scale=1.0)
                nc.scalar.activation(out=sr[:], in_=sr[:], func=Act.Exp,
                                     bias=0.0, scale=0.5)
                nc.vector.reciprocal(out=sr[:], in_=sr[:])   # 1/sqrt(n+eps)
                np1 = small.tile([64, NUM_CAPS], f32, tag="np1")
                nc.vector.tensor_scalar_add(out=np1[:], in0=n_t[:],
                                            scalar1=1.0)
                nc.vector.reciprocal(out=np1[:], in_=np1[:])  # 1/(1+n)
                fac = small.tile([64, NUM_CAPS], f32, tag="fac")
                nc.vector.tensor_mul(out=fac[:], in0=n_t[:], in1=np1[:])
                nc.vector.tensor_mul(out=fac[:], in0=fac[:], in1=sr[:])
                # v = s * fac (broadcast fac over the outer d dim)
                nc.vector.tensor_tensor(
                    out=vrep[0:64, :].rearrange("p (d o) -> p d o",
                                                d=DIM_CAPS),
                    in0=sq[:].rearrange("p (d o) -> p d o", d=DIM_CAPS),
                    in1=bass.AP(tensor=fac.tensor, offset=fac[:].offset,
                                ap=[list(fac[:].ap[0]), [0, DIM_CAPS],
                                    [1, NUM_CAPS]]),
                    op=Alu.mult)
                nc.sync.dma_start(vrep[64:128, :], vrep[0:64, :])

            if stage >= 1:
                sq = all_reduce(s0)
                squash_to_vrep(sq)
            if stage == 1:
                vr32 = work.tile([128, OD], f32, tag="vr32")
                nc.scalar.copy(out=vr32[:], in_=vrep[:])
                nc.sync.dma_start(out_d[:], vr32[:])

            # ---------------- iterations 1..2 -----------------------------
            last_it = ROUTING_ITERS if stage >= 3 else stage
            if stage == 4:
                last_it = ROUTING_ITERS
            with tc.tile_pool(name="ps", bufs=4, space="PSUM") as ps:
                N_QUADS = N_PAIRS // 2
                GRP = 4  # quads per accumulation group (bf16 add tree)
                for it in range(1, last_it):
                    nc.gpsimd.memset(acc2[:], 0.0)
                    nc.gpsimd.memset(acc2_g[:], 0.0)
                    cm_tiles = {0: [], 1: []}
                    for q in range(N_QUADS):
                        # a quad = 2 consecutive pairs (4 capsules);
                        # alternate quads between DVE and GPSIMD streams
                        side = q % 2
                        eng = nc.vector if side == 0 else nc.gpsimd
                        my_acc = acc2 if side == 0 else acc2_g
                        ub = work.tile([128, 2 * OD], bf16, tag="ub", bufs=8)
                        for sub in range(2):
                            pair = 2 * q + sub
                            rg, pj = pair % N_RG, pair // N_RG
                            ups = ps.tile([128, OD], f32, name="ups",
                                          tag="ups")
                            for h in range(2):
                                nc.tensor.matmul(
                                    ups[:, 512 * h:512 * h + 512],
                                    lhsT_of(xbd, rg, pj),
                                    rhs_of(wp, rg, pj, h),
                                    start=True, stop=True,
                                    tile_position=(32 * rg, 0),
                                )
                            # evacuate u to SBUF as bf16 on the scalar
                            # engine so DVE tensor ops run in 2x mode
                            nc.scalar.copy(out=ub[:, OD * sub:OD * (sub + 1)],
                                           in_=ups[:])
                        # agreement = sum_d u * v  (both pairs at once);
                        # free layout of u is (sub, d, o) -- d-major
                        m = work.tile([128, 2 * OD], bf16, tag="m", bufs=2)
                        nc.vector.tensor_tensor(
                            out=m[:].rearrange("p (s od) -> p s od", s=2),
                            in0=ub[:].rearrange("p (s od) -> p s od", s=2),
                            in1=bass.AP(tensor=vrep.tensor,
                                        offset=vrep[:].offset,
                                        ap=[list(vrep[:].ap[0]), [0, 2],
                                            [1, OD]]),
                            op=Alu.mult)
                        # two halvings over d (2x bf16 adds on contiguous
                        # d-blocks), then a strided reduce over the rest
                        mh = work.tile([128, OD], bf16, tag="mh", bufs=3)
                        mv = m[:].rearrange("p (s hd x) -> p s hd x",
                                            s=2, hd=2)
                        nc.vector.tensor_tensor(
                            out=mh[:].rearrange("p (s x) -> p s x", s=2),
                            in0=mv[:, :, 0, :], in1=mv[:, :, 1, :],
                            op=Alu.add)
                        mhv = mh[:].rearrange("p (s hd x) -> p s hd x",
                                              s=2, hd=2)
                        nc.vector.tensor_tensor(
                            out=mhv[:, :, 0, :], in0=mhv[:, :, 0, :],
                            in1=mhv[:, :, 1, :], op=Alu.add)
                        # reduce remaining 8 d-blocks: AP dims [s, o, d]
                        red_in = bass.AP(
                            tensor=mh.tensor, offset=mh[:].offset,
                            ap=[list(mh[:].ap[0]), [512, 2], [1, NUM_CAPS],
                                [NUM_CAPS, 8]])
                        bsl = bl[:, NUM_CAPS * 2 * q:NUM_CAPS * 2 * (q + 1)]
                        if it == 1:
                            # b was zero: logits = agreement, written directly
                            nc.vector.tensor_reduce(
                                out=bsl, in_=red_in, axis=AxX, op=Alu.add)
                        else:
                            agr = work.tile([128, 2 * NUM_CAPS], bf16,
                                            tag="agr", bufs=6)
                            nc.vector.tensor_reduce(
                                out=agr[:], in_=red_in, axis=AxX, op=Alu.add)
                            nc.vector.tensor_add(out=bsl, in0=bsl,
                                                 in1=agr[:])
                        # softmax over o (free dim); logits are small, so
                        # exp without max-subtraction is safe
                        ce = work.tile([128, 2 * NUM_CAPS], bf16, tag="ce", bufs=10)
                        zs = work.tile([128, 2], f32, tag="zs", bufs=10)
                        # per-pair exp with fused row-sum (accum_out) -- the
                        # softmax denominator comes for free on ACT
                        for sub in range(2):
                            nc.scalar.activation(
                                out=ce[:, NUM_CAPS * sub:NUM_CAPS * (sub + 1)],
                                in_=bsl[:, NUM_CAPS * sub:NUM_CAPS * (sub + 1)],
                                func=Act.Exp,
                                accum_out=zs[:, sub:sub + 1])
                        nc.vector.reciprocal(out=zs[:], in_=zs[:])
                        # c = e / Z  (broadcast 1/Z over o)
                        nc.vector.tensor_tensor(
                            out=ce[:].rearrange("p (s o) -> p s o", s=2),
                            in0=ce[:].rearrange("p (s o) -> p s o", s=2),
                            in1=bass.AP(tensor=zs.tensor, offset=zs[:].offset,
                                        ap=[list(zs[:].ap[0]), [1, 2],
                                            [0, NUM_CAPS]]),
                            op=Alu.mult)
                        # cm = c * u  (c broadcast over the outer d dim ->
                        # innermost step stays 1, keeps 2x mode)
                        csl = bass.AP(
                            tensor=ce.tensor, offset=ce[:].offset,
                            ap=[list(ce[:].ap[0]), [NUM_CAPS, 2],
                                [0, DIM_CAPS], [1, NUM_CAPS]])
                        cm = work.tile([128, 2 * OD], bf16, name="cm",
                                       tag="cm", bufs=8)
                        eng.tensor_tensor(
                            out=cm[:].rearrange("p (s d o) -> p s d o",
                                                s=2, d=DIM_CAPS),
                            in0=ub[:].rearrange("p (s d o) -> p s d o",
                                                s=2, d=DIM_CAPS),
                            in1=csl, op=Alu.mult)
                        cm_tiles[side].append(cm)
                        if len(cm_tiles[side]) == GRP:
                            c0, c1, c2, c3 = cm_tiles[side]
                            cm_tiles[side] = []
                            eng.tensor_add(out=c0[:], in0=c0[:], in1=c1[:])
                            eng.tensor_add(out=c2[:], in0=c2[:], in1=c3[:])
                            eng.tensor_add(out=c0[:], in0=c0[:], in1=c2[:])
                            eng.tensor_add(out=my_acc[:], in0=my_acc[:],
                                           in1=c0[:])
                    # fold acc2/acc2_g [128, 2*OD] f32 into acc [128, OD]
                    nc.vector.tensor_add(out=acc[:], in0=acc2[:, 0:OD],
                                         in1=acc2[:, OD:2 * OD])
                    nc.vector.tensor_add(out=acc[:], in0=acc[:],
                                         in1=acc2_g[:, 0:OD])
                    nc.vector.tensor_add(out=acc[:], in0=acc[:],
                                         in1=acc2_g[:, OD:2 * OD])
                    if it < last_it - 1:
                        sp = small.tile([64, OD], f32, tag="sfold")
                        tmpu2 = small.tile([64, OD], f32, tag="tmpu")
                        nc.sync.dma_start(tmpu2[:], acc[64:128, :])
                        nc.vector.tensor_add(out=sp[:], in0=acc[0:64, :],
                                             in1=tmpu2[:])
                        sq = all_reduce(sp)
                        squash_to_vrep(sq)
                    else:
                        nc.sync.dma_start(out_d[:], acc[:])
    nc.compile()
    return nc


def _prep_inputs(x, W):
    """Build per-core xbd [128, N_PJ*128] and wp [128, N_PJ*OD] arrays."""
    import ml_dtypes
    bf16 = ml_dtypes.bfloat16
    ins = []
    for c in range(N_CORES):
        xc = x[:, c * I_LOC:(c + 1) * I_LOC, :]          # [64, 256, 16]
        Wc = W[c * I_LOC:(c + 1) * I_LOC]                # [256, 32, 32, 16]
        # i_loc = 8*pj + 2*rg + ipar
        xr = np.ascontiguousarray(
            xc.reshape(B, N_PJ, N_RG, 2, IN_DIM)
              .transpose(3, 2, 4, 1, 0))                 # [ipar,rg,p,pj,b]
        xbd = np.zeros((N_RG, 2, IN_DIM, N_PJ, 2, B), dtype=np.float32)
        xbd[:, 0, :, :, 0, :] = xr[0]
        xbd[:, 1, :, :, 1, :] = xr[1]
        xbd = xbd.reshape(128, N_PJ * 128).astype(bf16)
        wr = np.ascontiguousarray(
            Wc.reshape(N_PJ, N_RG, 2, NUM_CAPS, DIM_CAPS, IN_DIM)
              .transpose(1, 2, 5, 0, 4, 3)               # [rg,ipar,p,pj,d,o]
              .reshape(128, N_PJ * OD)).astype(bf16)
        ins.append({"xbd": xbd, "wp": wr})
    return ins


def _squash_np(s):
    n = np.sum(np.square(s), axis=-1, keepdims=True)
    return (n / (1.0 + n)) * (s / np.sqrt(n + EPS))


def kernel(x, W, _trace=False):
    from concourse.bass_utils import run_bass_kernel_spmd

    x = np.asarray(x, dtype=np.float32)
    W = np.asarray(W, dtype=np.float32)
    if "nc" not in _CACHE:
        _CACHE["nc"] = _build_nc()
    nc = _CACHE["nc"]
    in_maps = _prep_inputs(x, W)
    res = run_bass_kernel_spmd(nc, in_maps, core_ids=list(range(N_CORES)),
                               trace=_trace)
    _CACHE["last_result"] = res
    sp = np.stack([r["out_sp2"] for r in res.results])   # [8, 128, OD]
    s2 = sp[:, 0:64, :].sum(axis=0) + sp[:, 64:128, :].sum(axis=0)
    s2_od = s2.reshape(B, DIM_CAPS, NUM_CAPS).transpose(0, 2, 1)
    v = _squash_np(np.ascontiguousarray(s2_od))
    return v.astype(np.float32)
